# revision 19
# baseline (speedup 1.0000x reference)
import sys

sys.path.insert(0, "/opt/trn_rl_repo")

import numpy as np
import ml_dtypes

import concourse.bass as bass
import concourse.bacc as bacc
import concourse.tile as tile
from concourse import mybir
from concourse import bass_utils

F32 = mybir.dt.float32
BF16 = mybir.dt.bfloat16
Alu = mybir.AluOpType
Act = mybir.ActivationFunctionType
npbf = ml_dtypes.bfloat16

B = 512
IN_F = 512
OUT_F = 64
KD = 16
NCORES = 8
JPC = B // NCORES          # 64 j-rows per core
NCH = 8                    # ok-chunks of 128 = (8 o) x (16 k)
DVE_CHUNKS = (0, 1, 2, 3, 4, 5)   # absdiff via max-trick (DVE or GpSimd)
GPS_CHUNKS = ()                   # GpSimd tensor_scalar measured ~8x too slow
ACT_CHUNKS = (6, 7)               # absdiff via ACT Abs

_CACHE = {}


def _build_nc():
    nc = bacc.Bacc("TRN2", target_bir_lowering=False)

    X = nc.dram_tensor("x_full", [B, IN_F], BF16, kind="ExternalInput")
    T2 = nc.dram_tensor("t2", [IN_F, OUT_F * KD], BF16, kind="ExternalInput")
    XJ = nc.dram_tensor("xj", [JPC, IN_F], BF16, kind="ExternalInput")
    WCOL = nc.dram_tensor("wcol", [128, 32], F32, kind="ExternalInput")
    BD1 = nc.dram_tensor("bd1", [NCH, 128, OUT_F], BF16, kind="ExternalInput")
    BD2 = nc.dram_tensor("bd2", [NCH, 128, OUT_F], BF16, kind="ExternalInput")
    NI2 = nc.dram_tensor("ni2", [OUT_F, 128], BF16, kind="ExternalInput")
    IDENT = nc.dram_tensor("ident", [128, 128], BF16, kind="ExternalInput")
    O = nc.dram_tensor("o", [JPC, OUT_F], F32, kind="ExternalOutput")

    with tile.TileContext(nc) as tc:
        with tc.tile_pool(name="consts", bufs=1) as consts:
            # ---- constant loads -------------------------------------------
            ident = consts.tile([128, 128], BF16, tag="ident")
            nc.gpsimd.dma_start(out=ident[:], in_=IDENT[:])
            bd1 = consts.tile([128, NCH, OUT_F], BF16, tag="bd1")
            nc.gpsimd.dma_start(out=bd1[:], in_=BD1.rearrange("c p m -> p c m"))
            bd2 = consts.tile([128, NCH, OUT_F], BF16, tag="bd2")
            nc.gpsimd.dma_start(out=bd2[:], in_=BD2.rearrange("c p m -> p c m"))
            wcol = consts.tile([128, 32], F32, tag="wcol")
            nc.gpsimd.dma_start(out=wcol[:], in_=WCOL[:])
            ni2 = consts.tile([OUT_F, 128], BF16, tag="ni2")
            nc.gpsimd.dma_start(out=ni2[:], in_=NI2[:])
            ones1 = consts.tile([1, 128], BF16, tag="ones1")
            nc.vector.memset(ones1[:], 1.0)
            big1 = consts.tile([1, 1], BF16, tag="big1")
            nc.vector.memset(big1[:], 60000.0)

            xj16 = consts.tile([JPC, IN_F], BF16, tag="xj16")
            nc.gpsimd.dma_start(out=xj16[:], in_=XJ[:])

            x16 = consts.tile([128, 4, IN_F], BF16, tag="x16")
            t216 = consts.tile([128, 4, OUT_F * KD], BF16, tag="t216")
            Xr = X.rearrange("(c p) f -> p c f", p=128)
            T2r = T2.rearrange("(c p) n -> p c n", p=128)
            for fc in range(4):
                nc.gpsimd.dma_start(out=t216[:, fc, :], in_=T2r[:, fc, :])
                nc.gpsimd.dma_start(out=x16[:, fc, :], in_=Xr[:, fc, :])

            # ---- transpose x (bf16) via PE: xT[fp, fc, i] = x[i, 128*fc+fp]
            xT = consts.tile([128, 4, B], BF16, tag="xT")
            xjT = consts.tile([128, 4, JPC], BF16, tag="xjT")
            with tc.tile_pool(name="pst", bufs=2, space="PSUM") as pst:
                for fc in range(4):
                    for bc in range(4):
                        pt = pst.tile([128, 128], BF16, tag="pt")
                        nc.tensor.transpose(
                            pt[:], x16[:, bc, fc * 128:(fc + 1) * 128], ident[:]
                        )
                        nc.vector.tensor_copy(
                            xT[:, fc, bc * 128:(bc + 1) * 128], pt[:]
                        )
                    ptj = pst.tile([128, JPC], BF16, tag="ptj")
                    nc.tensor.transpose(
                        ptj[:], xj16[:, fc * 128:(fc + 1) * 128], ident[0:64, 0:64]
                    )
                    nc.vector.tensor_copy(xjT[:, fc, :], ptj[:])

            # ---- MT projection: MT[ok, i] = sum_f T2[f, ok] * x[i, f] ----
            MT = consts.tile([128, NCH, B], BF16, tag="MT")
            s16 = consts.tile([128, NCH, JPC], BF16, tag="s16")
            sf32 = consts.tile([128, NCH, JPC], F32, tag="sf32")
            negs = consts.tile([128, NCH, JPC], F32, tag="negs")
            with tc.tile_pool(name="psp", bufs=2, space="PSUM") as psp, \
                 tc.tile_pool(name="psq", bufs=1, space="PSUM") as psq:
                for m in range(NCH):
                    pm = psp.tile([128, B], F32, tag="pm")
                    for fc in range(4):
                        nc.tensor.matmul(
                            pm[:],
                            t216[:, fc, m * 128:(m + 1) * 128],
                            xT[:, fc, :],
                            start=(fc == 0),
                            stop=(fc == 3),
                        )
                    nc.scalar.copy(MT[:, m, :], pm[:])
                for m in range(NCH):
                    pj = psq.tile([128, JPC], F32, tag="pj")
                    for fc in range(4):
                        nc.tensor.matmul(
                            pj[:],
                            t216[:, fc, m * 128:(m + 1) * 128],
                            xjT[:, fc, :],
                            start=(fc == 0),
                            stop=(fc == 3),
                        )
                    nc.vector.tensor_copy(s16[:, m, :], pj[:])
                    nc.vector.tensor_copy(sf32[:, m, :], s16[:, m, :])
                    nc.vector.tensor_scalar(
                        negs[:, m, :], s16[:, m, :], -1.0, None, Alu.mult
                    )

                # ---- A_sum variants: sum_k MT over first 5 / 6 chunks ----
                asbs = {}
                csbs = {}
                cbiases = {}
                for nd in (5, 6):
                    pa = psq.tile([64, B], F32, tag=f"pa{nd}", name=f"pa{nd}")
                    for ci in range(nd):
                        nc.tensor.matmul(
                            pa[:], bd1[:, ci, :], MT[:, ci, :],
                            start=(ci == 0), stop=(ci == nd - 1),
                        )
                    asb = consts.tile([64, B], BF16, tag=f"asb{nd}", name=f"asb{nd}")
                    nc.scalar.copy(asb[:], pa[:])
                    asbs[nd] = asb
                    pcn = psq.tile([64, JPC], F32, tag=f"pc{nd}", name=f"pc{nd}")
                    for ci in range(nd):
                        nc.tensor.matmul(
                            pcn[:], bd1[:, ci, :], s16[:, ci, :],
                            start=(ci == 0), stop=(ci == nd - 1),
                        )
                    csb = consts.tile([64, JPC], F32, tag=f"csb{nd}", name=f"csb{nd}")
                    nc.vector.tensor_copy(csb[:], pcn[:])
                    cbias = consts.tile([128, 32], F32, tag=f"cbias{nd}", name=f"cbias{nd}")
                    nc.gpsimd.dma_start(out=cbias[0:64, :], in_=csb[:, 0:64:2])
                    nc.gpsimd.dma_start(out=cbias[64:128, :], in_=csb[:, 1:64:2])
                    cbiases[nd] = cbias

            # ---- main loop ------------------------------------------------
            svals = consts.tile([128, 32], F32, tag="svals")
            with tc.tile_pool(name="adp", bufs=20) as adp, \
                 tc.tile_pool(name="psn", bufs=3, space="PSUM") as psn, \
                 tc.tile_pool(name="eop", bufs=3) as eop:
                for jp in range(32):
                    nd = 5 if jp % 5 == 0 else 6   # 7 jpairs shed chunk 5 to ACT
                    np_ps = psn.tile([128, B], F32, tag="np_ps")
                    for h in range(2):
                        jl = 2 * jp + h
                        for ci, c in enumerate(DVE_CHUNKS + ACT_CHUNKS):
                            ad = adp.tile([128, B], BF16, tag="ad")
                            if c in ACT_CHUNKS or c >= nd:
                                nc.scalar.activation(
                                    ad[:], MT[:, c, :], Act.Abs,
                                    bias=negs[:, c, jl:jl + 1], scale=1.0,
                                )
                            else:
                                nc.vector.tensor_scalar(
                                    ad[:], MT[:, c, :], sf32[:, c, jl:jl + 1],
                                    None, Alu.max,
                                )
                            lhs = bd1[:, c, :] if (c in ACT_CHUNKS or c >= nd) else bd2[:, c, :]
                            nc.tensor.matmul(
                                np_ps[h * 64:(h + 1) * 64, :], lhs, ad[:],
                                start=(ci == 0), stop=False,
                            )
                    # kill the diagonal by adding 60000 into the diag cells
                    # (i-axis reordered: own strip first -> diag col = j_local)
                    for h in range(2):
                        col = 2 * jp + h
                        nc.tensor.matmul(
                            np_ps[h * 64:(h + 1) * 64, col:col + 1],
                            ones1[0:1, 0:64], big1[0:1, 0:1],
                            start=False, stop=False,
                        )
                    # subtract A_sum on both halves in one matmul
                    nc.tensor.matmul(
                        np_ps[:], ni2[:], asbs[nd][:], start=False, stop=True,
                    )
                    eo = eop.tile([128, B], BF16, tag="eo")
                    nc.scalar.activation(
                        eo[:], np_ps[:], Act.Exp,
                        bias=cbiases[nd][:, jp:jp + 1], scale=-1.0,
                        accum_out=svals[:, jp:jp + 1],
                    )

                # ---- finalize: res = svals * wcol (diag excluded above) --
                resf = consts.tile([128, 32], F32, tag="resf")
                nc.vector.tensor_tensor(
                    resf[:], svals[:], wcol[:], Alu.mult
                )
                oap = O.ap()
                nc.gpsimd.dma_start(
                    out=bass.AP(tensor=oap.tensor, offset=0,
                                ap=[[1, 64], [128, 32]]),
                    in_=resf[0:64, :],
                )
                nc.gpsimd.dma_start(
                    out=bass.AP(tensor=oap.tensor, offset=64,
                                ap=[[1, 64], [128, 32]]),
                    in_=resf[64:128, :],
                )

    nc.compile()
    return nc


def _host_consts():
    bd1 = np.zeros((NCH, 128, OUT_F), dtype=npbf)
    for c in range(NCH):
        for ol in range(8):
            bd1[c, ol * 16:(ol + 1) * 16, 8 * c + ol] = 1.0
    bd2 = (bd1.astype(np.float32) * 2.0).astype(npbf)
    ni2 = np.zeros((OUT_F, 128), dtype=np.float32)
    for m in range(128):
        ni2[m % 64, m] = -1.0
    ni2 = ni2.astype(npbf)
    ident = np.eye(128, dtype=np.float32).astype(npbf)
    return bd1, bd2, ni2, ident


def make_in_maps(x, w, t2, bd1, bd2, ni2, ident):
    in_maps = []
    for c in range(NCORES):
        wcol = np.empty((128, 32), np.float32)
        jidx = 64 * c + 2 * np.arange(32)
        wcol[0:64, :] = w[0, jidx][None, :]
        wcol[64:128, :] = w[0, jidx + 1][None, :]
        # i-axis reorder: own j-strip first, so the diagonal for local j
        # sits at column j_local (static position for the diag kill)
        xr = np.concatenate(
            [x[64 * c:64 * (c + 1), :],
             x[:64 * c, :],
             x[64 * (c + 1):, :]], axis=0)
        in_maps.append({
            "x_full": np.ascontiguousarray(xr.astype(npbf)),
            "t2": np.ascontiguousarray(t2.astype(npbf)),
            "xj": np.ascontiguousarray(x[64 * c:64 * (c + 1), :].astype(npbf)),
            "wcol": wcol,
            "bd1": bd1,
            "bd2": bd2,
            "ni2": ni2,
            "ident": ident,
        })
    return in_maps


def kernel(x, w, T):
    x = np.ascontiguousarray(np.asarray(x, dtype=np.float32))
    w = np.ascontiguousarray(np.asarray(w, dtype=np.float32)).reshape(1, B)
    T = np.asarray(T, dtype=np.float32)
    t2 = np.ascontiguousarray(T.reshape(IN_F, OUT_F * KD))

    if "nc" not in _CACHE:
        _CACHE["nc"] = _build_nc()
        _CACHE["consts"] = _host_consts()
    nc = _CACHE["nc"]
    bd1, bd2, ni2, ident = _CACHE["consts"]

    in_maps = make_in_maps(x, w, t2, bd1, bd2, ni2, ident)

    res = bass_utils.run_bass_kernel_spmd(nc, in_maps, core_ids=list(range(NCORES)))
    out = np.concatenate([res.results[c]["o"] for c in range(NCORES)], axis=0)
    return out.astype(np.float32)


if __name__ == "__main__":
    rng = np.random.default_rng(0)
    x = rng.standard_normal((B, IN_F)).astype(np.float32)
    w = rng.random((1, B)).astype(np.float32)
    T = rng.standard_normal((IN_F, OUT_F, KD)).astype(np.float32)
    out = kernel(x=x, w=w, T=T)
    print("out", out.shape, out.dtype, "nonzero:", np.count_nonzero(out),
          "maxabs:", np.abs(out).max())


# revision 20
# speedup vs baseline: 1.1279x; 1.1279x over previous
import sys

sys.path.insert(0, "/opt/trn_rl_repo")

import numpy as np
import ml_dtypes

import concourse.bass as bass
import concourse.bacc as bacc
import concourse.tile as tile
from concourse import mybir
from concourse import bass_utils

F32 = mybir.dt.float32
BF16 = mybir.dt.bfloat16
Alu = mybir.AluOpType
Act = mybir.ActivationFunctionType
npbf = ml_dtypes.bfloat16

B = 512
IN_F = 512
OUT_F = 64
KD = 16
NCORES = 8
JPC = B // NCORES          # 64 j-rows per core
NCH = 8                    # ok-chunks of 128 = (8 o) x (16 k)
DVE_CHUNKS = (0, 1, 2, 3, 4, 5)   # absdiff via max-trick (DVE or GpSimd)
GPS_CHUNKS = ()                   # GpSimd tensor_scalar measured ~8x too slow
ACT_CHUNKS = (6, 7)               # absdiff via ACT Abs

_CACHE = {}


def _build_nc():
    nc = bacc.Bacc("TRN2", target_bir_lowering=False)

    X = nc.dram_tensor("x_full", [B, IN_F], BF16, kind="ExternalInput")
    T2 = nc.dram_tensor("t2", [IN_F, OUT_F * KD], BF16, kind="ExternalInput")
    XJ = nc.dram_tensor("xj", [JPC, IN_F], BF16, kind="ExternalInput")
    WCOL = nc.dram_tensor("wcol", [128, 32], F32, kind="ExternalInput")
    BD1 = nc.dram_tensor("bd1", [NCH, 128, OUT_F], BF16, kind="ExternalInput")
    BD2 = nc.dram_tensor("bd2", [NCH, 128, OUT_F], BF16, kind="ExternalInput")
    NI2 = nc.dram_tensor("ni2", [OUT_F, 128], BF16, kind="ExternalInput")
    IDENT = nc.dram_tensor("ident", [128, 128], BF16, kind="ExternalInput")
    O = nc.dram_tensor("o", [JPC, OUT_F], F32, kind="ExternalOutput")

    with tile.TileContext(nc) as tc:
        with tc.tile_pool(name="consts", bufs=1) as consts:
            # ---- constant loads -------------------------------------------
            ident = consts.tile([128, 128], BF16, tag="ident")
            nc.gpsimd.dma_start(out=ident[:], in_=IDENT[:])
            bd1 = consts.tile([128, NCH, OUT_F], BF16, tag="bd1")
            nc.gpsimd.dma_start(out=bd1[:], in_=BD1.rearrange("c p m -> p c m"))
            bd2 = consts.tile([128, NCH, OUT_F], BF16, tag="bd2")
            nc.gpsimd.dma_start(out=bd2[:], in_=BD2.rearrange("c p m -> p c m"))
            wcol = consts.tile([128, 32], F32, tag="wcol")
            nc.gpsimd.dma_start(out=wcol[:], in_=WCOL[:])
            ni2 = consts.tile([OUT_F, 128], BF16, tag="ni2")
            nc.gpsimd.dma_start(out=ni2[:], in_=NI2[:])
            ones1 = consts.tile([1, 128], BF16, tag="ones1")
            nc.vector.memset(ones1[:], 1.0)
            big1 = consts.tile([1, 1], BF16, tag="big1")
            nc.vector.memset(big1[:], 60000.0)

            xj16 = consts.tile([JPC, IN_F], BF16, tag="xj16")
            nc.gpsimd.dma_start(out=xj16[:], in_=XJ[:])

            x16 = consts.tile([128, 4, IN_F], BF16, tag="x16")
            t216 = consts.tile([128, 4, OUT_F * KD], BF16, tag="t216")
            Xr = X.rearrange("(c p) f -> p c f", p=128)
            T2r = T2.rearrange("(c p) n -> p c n", p=128)
            for fc in range(4):
                nc.gpsimd.dma_start(out=t216[:, fc, :], in_=T2r[:, fc, :])
                nc.gpsimd.dma_start(out=x16[:, fc, :], in_=Xr[:, fc, :])

            # ---- transpose x (bf16) via PE: xT[fp, fc, i] = x[i, 128*fc+fp]
            xT = consts.tile([128, 4, B], BF16, tag="xT")
            xjT = consts.tile([128, 4, JPC], BF16, tag="xjT")
            with tc.tile_pool(name="pst", bufs=2, space="PSUM") as pst:
                for fc in range(4):
                    for bc in range(4):
                        pt = pst.tile([128, 128], BF16, tag="pt")
                        nc.tensor.transpose(
                            pt[:], x16[:, bc, fc * 128:(fc + 1) * 128], ident[:]
                        )
                        nc.vector.tensor_copy(
                            xT[:, fc, bc * 128:(bc + 1) * 128], pt[:]
                        )
                    ptj = pst.tile([128, JPC], BF16, tag="ptj")
                    nc.tensor.transpose(
                        ptj[:], xj16[:, fc * 128:(fc + 1) * 128], ident[0:64, 0:64]
                    )
                    nc.vector.tensor_copy(xjT[:, fc, :], ptj[:])

            # ---- MT projection: MT[ok, i] = sum_f T2[f, ok] * x[i, f] ----
            MT = consts.tile([128, NCH, B], BF16, tag="MT")
            s16 = consts.tile([128, NCH, JPC], BF16, tag="s16")
            sf32 = consts.tile([128, NCH, JPC], F32, tag="sf32")
            negs = consts.tile([128, NCH, JPC], F32, tag="negs")
            with tc.tile_pool(name="psp", bufs=2, space="PSUM") as psp, \
                 tc.tile_pool(name="psq", bufs=1, space="PSUM") as psq:
                for m in range(NCH):
                    pm = psp.tile([128, B], F32, tag="pm")
                    for fc in range(4):
                        nc.tensor.matmul(
                            pm[:],
                            t216[:, fc, m * 128:(m + 1) * 128],
                            xT[:, fc, :],
                            start=(fc == 0),
                            stop=(fc == 3),
                        )
                    nc.scalar.copy(MT[:, m, :], pm[:])
                for m in range(NCH):
                    pj = psq.tile([128, JPC], F32, tag="pj")
                    for fc in range(4):
                        nc.tensor.matmul(
                            pj[:],
                            t216[:, fc, m * 128:(m + 1) * 128],
                            xjT[:, fc, :],
                            start=(fc == 0),
                            stop=(fc == 3),
                        )
                    nc.vector.tensor_copy(s16[:, m, :], pj[:])
                    nc.vector.tensor_copy(sf32[:, m, :], s16[:, m, :])
                    nc.vector.tensor_scalar(
                        negs[:, m, :], s16[:, m, :], -1.0, None, Alu.mult
                    )

                # ---- A_sum variants: sum_k MT over first 5 / 6 chunks ----
                asbs = {}
                csbs = {}
                cbiases = {}
                for nd in (5, 6):
                    pa = psq.tile([64, B], F32, tag=f"pa{nd}", name=f"pa{nd}")
                    for ci in range(nd):
                        nc.tensor.matmul(
                            pa[:], bd1[:, ci, :], MT[:, ci, :],
                            start=(ci == 0), stop=(ci == nd - 1),
                        )
                    asb = consts.tile([64, B], BF16, tag=f"asb{nd}", name=f"asb{nd}")
                    nc.scalar.copy(asb[:], pa[:])
                    asbs[nd] = asb
                    pcn = psq.tile([64, JPC], F32, tag=f"pc{nd}", name=f"pc{nd}")
                    for ci in range(nd):
                        nc.tensor.matmul(
                            pcn[:], bd1[:, ci, :], s16[:, ci, :],
                            start=(ci == 0), stop=(ci == nd - 1),
                        )
                    csb = consts.tile([64, JPC], F32, tag=f"csb{nd}", name=f"csb{nd}")
                    nc.vector.tensor_copy(csb[:], pcn[:])
                    cbias = consts.tile([128, 32], F32, tag=f"cbias{nd}", name=f"cbias{nd}")
                    nc.gpsimd.dma_start(out=cbias[0:64, :], in_=csb[:, 0:64:2])
                    nc.gpsimd.dma_start(out=cbias[64:128, :], in_=csb[:, 1:64:2])
                    cbiases[nd] = cbias

            # ---- main loop ------------------------------------------------
            svals = consts.tile([128, 32], F32, tag="svals")
            with tc.tile_pool(name="adp", bufs=20) as adp, \
                 tc.tile_pool(name="psn", bufs=3, space="PSUM") as psn, \
                 tc.tile_pool(name="eop", bufs=3) as eop:
                for jp in range(32):
                    nd = 6   # ACT absdiff too expensive to absorb more (measured)
                    np_ps = psn.tile([128, B], F32, tag="np_ps")
                    for h in range(2):
                        jl = 2 * jp + h
                        for ci, c in enumerate(DVE_CHUNKS + ACT_CHUNKS):
                            ad = adp.tile([128, B], BF16, tag="ad")
                            if c in ACT_CHUNKS or c >= nd:
                                nc.scalar.activation(
                                    ad[:], MT[:, c, :], Act.Abs,
                                    bias=negs[:, c, jl:jl + 1], scale=1.0,
                                )
                            else:
                                nc.vector.tensor_scalar(
                                    ad[:], MT[:, c, :], sf32[:, c, jl:jl + 1],
                                    None, Alu.max,
                                )
                            lhs = bd1[:, c, :] if (c in ACT_CHUNKS or c >= nd) else bd2[:, c, :]
                            nc.tensor.matmul(
                                np_ps[h * 64:(h + 1) * 64, :], lhs, ad[:],
                                start=(ci == 0), stop=False,
                            )
                    # kill the diagonal by adding 60000 into the diag cells
                    # (i-axis reordered: own strip first -> diag col = j_local)
                    for h in range(2):
                        col = 2 * jp + h
                        nc.tensor.matmul(
                            np_ps[h * 64:(h + 1) * 64, col:col + 1],
                            ones1[0:1, 0:64], big1[0:1, 0:1],
                            start=False, stop=False,
                        )
                    # subtract A_sum on both halves in one matmul
                    nc.tensor.matmul(
                        np_ps[:], ni2[:], asbs[nd][:], start=False, stop=True,
                    )
                    eo = eop.tile([128, B], BF16, tag="eo")
                    nc.scalar.activation(
                        eo[:], np_ps[:], Act.Exp,
                        bias=cbiases[nd][:, jp:jp + 1], scale=-1.0,
                        accum_out=svals[:, jp:jp + 1],
                    )

                # ---- finalize: res = svals * wcol (diag excluded above) --
                resf = consts.tile([128, 32], F32, tag="resf")
                nc.vector.tensor_tensor(
                    resf[:], svals[:], wcol[:], Alu.mult
                )
                oap = O.ap()
                nc.gpsimd.dma_start(
                    out=bass.AP(tensor=oap.tensor, offset=0,
                                ap=[[1, 64], [128, 32]]),
                    in_=resf[0:64, :],
                )
                nc.gpsimd.dma_start(
                    out=bass.AP(tensor=oap.tensor, offset=64,
                                ap=[[1, 64], [128, 32]]),
                    in_=resf[64:128, :],
                )

    nc.compile()
    return nc


def _host_consts():
    bd1 = np.zeros((NCH, 128, OUT_F), dtype=npbf)
    for c in range(NCH):
        for ol in range(8):
            bd1[c, ol * 16:(ol + 1) * 16, 8 * c + ol] = 1.0
    bd2 = (bd1.astype(np.float32) * 2.0).astype(npbf)
    ni2 = np.zeros((OUT_F, 128), dtype=np.float32)
    for m in range(128):
        ni2[m % 64, m] = -1.0
    ni2 = ni2.astype(npbf)
    ident = np.eye(128, dtype=np.float32).astype(npbf)
    return bd1, bd2, ni2, ident


def make_in_maps(x, w, t2, bd1, bd2, ni2, ident):
    in_maps = []
    for c in range(NCORES):
        wcol = np.empty((128, 32), np.float32)
        jidx = 64 * c + 2 * np.arange(32)
        wcol[0:64, :] = w[0, jidx][None, :]
        wcol[64:128, :] = w[0, jidx + 1][None, :]
        # i-axis reorder: own j-strip first, so the diagonal for local j
        # sits at column j_local (static position for the diag kill)
        xr = np.concatenate(
            [x[64 * c:64 * (c + 1), :],
             x[:64 * c, :],
             x[64 * (c + 1):, :]], axis=0)
        in_maps.append({
            "x_full": np.ascontiguousarray(xr.astype(npbf)),
            "t2": np.ascontiguousarray(t2.astype(npbf)),
            "xj": np.ascontiguousarray(x[64 * c:64 * (c + 1), :].astype(npbf)),
            "wcol": wcol,
            "bd1": bd1,
            "bd2": bd2,
            "ni2": ni2,
            "ident": ident,
        })
    return in_maps


def kernel(x, w, T):
    x = np.ascontiguousarray(np.asarray(x, dtype=np.float32))
    w = np.ascontiguousarray(np.asarray(w, dtype=np.float32)).reshape(1, B)
    T = np.asarray(T, dtype=np.float32)
    t2 = np.ascontiguousarray(T.reshape(IN_F, OUT_F * KD))

    if "nc" not in _CACHE:
        _CACHE["nc"] = _build_nc()
        _CACHE["consts"] = _host_consts()
    nc = _CACHE["nc"]
    bd1, bd2, ni2, ident = _CACHE["consts"]

    in_maps = make_in_maps(x, w, t2, bd1, bd2, ni2, ident)

    res = bass_utils.run_bass_kernel_spmd(nc, in_maps, core_ids=list(range(NCORES)))
    out = np.concatenate([res.results[c]["o"] for c in range(NCORES)], axis=0)
    return out.astype(np.float32)


if __name__ == "__main__":
    rng = np.random.default_rng(0)
    x = rng.standard_normal((B, IN_F)).astype(np.float32)
    w = rng.random((1, B)).astype(np.float32)
    T = rng.standard_normal((IN_F, OUT_F, KD)).astype(np.float32)
    out = kernel(x=x, w=w, T=T)
    print("out", out.shape, out.dtype, "nonzero:", np.count_nonzero(out),
          "maxabs:", np.abs(out).max())
